# revision 3
# baseline (speedup 1.0000x reference)
"""Trainium2 Bass kernel for nn_CustomConv1D_d (rank-1 dense conv1d, stride 21).

Math: out[b, t, o] = r[b, t] for all o in [0, 237), where
  r[b, t] = sum_k w[k] * sum_c x[b, 21 t + k, c],  w = softmax(p3*i + p4*i^2).

Strategy (pure data parallel over batch, 4 batches per core):
  - Per core, view x as flat [43008, 237] (rows contiguous across batches;
    group boundaries align since 10752 = 21*512).
  - Stage 1 (DVE): tiles of 126 rows (= 6 tap-groups) x 237 channels; big
    DMAs load 16 row-blocks at once; one segmented reduce per big tile
    produces per-row channel sums s[126, ...].
  - Stage 2 (PE): one tiny matmul with a host-built block-diagonal weight
    matrix W6[126, 6] (W6[p, m] = w[p - 21 m]) turns row sums into group
    results r[6, 341] (+ a 42-row tail for the last 2 groups).
  - Stage 3: scatter r to a DRAM scratch [2048], gather back as [128, 16],
    broadcast each column across 237 channels (ACT engine), DMA out.
"""

import numpy as np
from contextlib import ExitStack

import concourse.bass as bass
import concourse.tile as tile
import concourse.mybir as mybir
from concourse.bass_utils import run_bass_kernel_spmd
from concourse.vector_clock import ScopedClock
from concourse._compat import not_none as nn

TAPS = 21
C = 237
B = 32
L = 10752
T = 512
NCORES = 8
BPC = B // NCORES            # 4 batches per core
ROWS = BPC * L               # 43008 rows per core
GROUPS = BPC * T             # 2048 groups per core
P = 126                      # rows per block = 6 groups (126 = 6*21)
GP = P // TAPS               # 6 groups per block
NBLK = ROWS // P             # 341 full blocks
TAIL_ROWS = ROWS - NBLK * P  # 42
TAIL_G = TAIL_ROWS // TAPS   # 2
R = 16                       # row-blocks per big DMA tile
NBIG, RREM = divmod(NBLK, R)  # 21 full big tiles, 5 leftover blocks
QCOLS = GROUPS // 128        # 16
OBLK = 4                     # q-blocks per output tile
F32 = mybir.dt.float32


class _TileContext(tile.TileContext):
    """TileContext with a post-scheduling pass that splits instructions
    carrying >1 sem wait onto preceding single-wait nops on the same
    engine — the pinned neuronxcc rejects instructions with multiple
    sync wait commands."""

    def schedule_and_allocate(self):
        ret = super().schedule_and_allocate()
        self._split_multi_waits()
        return ret

    def _split_multi_waits(self):
        nc = self.nc
        for fn in nc.m.functions:
            for bb in fn.blocks:
                if not any(
                    inst.sync_info
                    and inst.sync_info.on_wait
                    and len(inst.sync_info.on_wait) > 1
                    for inst in bb.instructions
                ):
                    continue
                new_insts = []
                for inst in bb.instructions:
                    si = inst.sync_info
                    waits = list(si.on_wait) if si and si.on_wait else []
                    if len(waits) > 1:
                        si.on_wait = waits[-1:]
                        for w in waits[:-1]:
                            nop = mybir.InstNoOp(
                                name=f"I-splitw-{nc.next_id()}",
                                engine=inst.engine,
                                sync_info=mybir.SyncInfo(on_wait=[w], on_update=[]),
                            )
                            nc.register_instruction(nop, overwrite=True)
                            new_insts.append(nop)
                    new_insts.append(inst)
                bb.instructions[:] = new_insts


def _build():
    nc = bass.Bass("TRN2", target_bir_lowering=False, debug=False)
    x = nc.dram_tensor("x", [ROWS, C], F32, kind="ExternalInput").ap()
    w6 = nc.dram_tensor("w6", [P, GP], F32, kind="ExternalInput").ap()
    y = nc.dram_tensor("y", [GROUPS, C], F32, kind="ExternalOutput").ap()
    r_dram = nc.dram_tensor("r_scratch", [GROUPS], F32).ap()

    with _TileContext(nc) as tc:
        with ExitStack() as ctx:
            xin = ctx.enter_context(tc.tile_pool(name="xin", bufs=6))
            sp = ctx.enter_context(tc.tile_pool(name="sp", bufs=1))
            op = ctx.enter_context(tc.tile_pool(name="op", bufs=2))
            pp = ctx.enter_context(tc.tile_pool(name="pp", bufs=1, space="PSUM"))

            w_sb = sp.tile([P, GP], F32)
            nc.gpsimd.dma_start(w_sb[:], w6)
            s_all = sp.tile([P, NBLK + 1], F32)

            # Stage 1: load + per-row channel sums.
            for i in range(NBIG + 1):
                j0 = i * R
                nb = R if i < NBIG else RREM
                if nb == 0:
                    break
                xt = xin.tile([P, R * C], F32, tag="xt")
                view = xt[:, 0 : nb * C].rearrange("p (r c) -> p r c", c=C)
                nc.sync.dma_start(
                    view,
                    x[j0 * P : (j0 + nb) * P, :].rearrange("(r p) c -> p r c", p=P),
                )
                nc.vector.reduce_sum(
                    s_all[:, j0 : j0 + nb], view, axis=mybir.AxisListType.X
                )
            # 42-row tail (last 2 groups).
            xtt = xin.tile([P, R * C], F32, tag="xt")
            nc.sync.dma_start(xtt[0:TAIL_ROWS, 0:C], x[NBLK * P : ROWS, :])
            nc.vector.reduce_sum(
                s_all[0:TAIL_ROWS, NBLK : NBLK + 1],
                xtt[0:TAIL_ROWS, 0:C].rearrange("p (r c) -> p r c", c=C),
                axis=mybir.AxisListType.X,
            )

            # Stage 2: combine taps via matmul.
            pr = pp.tile([GP, NBLK], F32)
            nc.tensor.matmul(pr[:], w_sb[:], s_all[:, 0:NBLK], start=True, stop=True)
            pr2 = pp.tile([TAIL_G, 1], F32)
            nc.tensor.matmul(
                pr2[:],
                w_sb[0:TAIL_ROWS, 0:TAIL_G],
                s_all[0:TAIL_ROWS, NBLK : NBLK + 1],
                start=True,
                stop=True,
            )

            # Stage 3: scatter r values to DRAM scratch, regather, broadcast.
            r_sb = sp.tile([GP, NBLK], F32)
            nc.vector.tensor_copy(r_sb[:], pr[:])
            r_sb2 = sp.tile([TAIL_G, 1], F32)
            nc.vector.tensor_copy(r_sb2[:], pr2[:])
            nc.gpsimd.dma_start(
                r_dram[0 : GP * NBLK].rearrange("(j m) -> m j", m=GP), r_sb[:]
            )
            nc.gpsimd.dma_start(
                r_dram[GP * NBLK : GROUPS].rearrange("(m o) -> m o", o=1), r_sb2[:]
            )

            r16 = sp.tile([128, QCOLS], F32)
            nc.gpsimd.dma_start(r16[:], r_dram.rearrange("(q p) -> p q", p=128))

            for ot in range(QCOLS // OBLK):
                osb = op.tile([128, OBLK * C], F32, tag="osb")
                nc.scalar.activation(
                    osb.rearrange("p (q c) -> p q c", c=C),
                    r16[:, ot * OBLK : (ot + 1) * OBLK, None].broadcast_to(
                        [128, OBLK, C]
                    ),
                    mybir.ActivationFunctionType.Identity,
                )
                nc.sync.dma_start(
                    y[ot * OBLK * 128 : (ot + 1) * OBLK * 128, :].rearrange(
                        "(q p) c -> p q c", p=128
                    ),
                    osb.rearrange("p (q c) -> p q c", c=C),
                )
    return nc


_NC_CACHE = {}


def _get_nc():
    if "nc" not in _NC_CACHE:
        _NC_CACHE["nc"] = _build()
    return _NC_CACHE["nc"]


def _weights(param3: float, param4: float) -> np.ndarray:
    i = np.arange(1, TAPS + 1, dtype=np.float32)
    logits = (np.float32(param3) * i + np.float32(param4) * i * i).astype(np.float32)
    e = np.exp(logits - logits.max(), dtype=np.float32)
    w = (e / e.sum()).astype(np.float32)
    w6 = np.zeros((P, GP), dtype=np.float32)
    for m in range(GP):
        w6[m * TAPS : (m + 1) * TAPS, m] = w
    return w6


def run_with_results(inputs, **spmd_kwargs):
    x = np.ascontiguousarray(np.asarray(inputs["inputs"], dtype=np.float32))
    assert x.shape == (B, L, C), x.shape
    w6 = _weights(
        float(np.asarray(inputs["param3"])), float(np.asarray(inputs["param4"]))
    )
    xs = x.reshape(NCORES, ROWS, C)
    in_maps = [{"x": xs[i], "w6": w6} for i in range(NCORES)]
    res = run_bass_kernel_spmd(_get_nc(), in_maps, list(range(NCORES)), **spmd_kwargs)
    out = np.stack([res.results[i]["y"] for i in range(NCORES)])
    return out.reshape(B, T, C).astype(np.float32, copy=False), res


def kernel(**inputs) -> np.ndarray:
    out, _ = run_with_results(inputs)
    return out


# revision 7
# speedup vs baseline: 1.4779x; 1.4779x over previous
"""Trainium2 Bass kernel for nn_CustomConv1D_d (rank-1 dense conv1d, stride 21).

Math: out[b, t, o] = r[b, t] for all o in [0, 237), where
  r[b, t] = sum_k w[k] * sum_c x[b, 21 t + k, c],  w = softmax(p3*i + p4*i^2).

Strategy (pure data parallel over batch, 4 batches per core):
  - Per core, view x as flat [43008, 237]; each output group t owns 21
    consecutive rows = 19908 contiguous bytes. Load tiles [128 groups,
    21*237] — one fully-contiguous 19908B DMA descriptor per partition.
  - Per tile, DVE does a segmented reduce over channels -> [128, 21]
    per-tap sums, multiplies by the tap weights, and reduces over taps
    -> r[128, 1] — already in output layout.
  - ACT engine broadcasts each r column across 237 channels; ACT-issued
    DMAs stream results out without stalling the input DMA ring.
"""

import numpy as np
from contextlib import ExitStack

import concourse.bass as bass
import concourse.tile as tile
import concourse.mybir as mybir
from concourse.bass_utils import run_bass_kernel_spmd

TAPS = 21
C = 237
B = 32
L = 10752
T = 512
NCORES = 8
BPC = B // NCORES            # 4 batches per core
ROWS = BPC * L               # 43008 rows per core
GROUPS = BPC * T             # 2048 groups per core
NQ = GROUPS // 128           # 16 tiles of 128 groups
GROUP_ROWS = 128 * TAPS      # 2688 input rows per tile
FD = TAPS * C                # 4977 elements per group
OBLK = 4                     # group-tiles per output tile
F32 = mybir.dt.float32


class _TileContext(tile.TileContext):
    """TileContext with a post-scheduling pass that splits instructions
    carrying >1 sem wait onto preceding single-wait nops on the same
    engine — the pinned neuronxcc rejects instructions with multiple
    sync wait commands."""

    def schedule_and_allocate(self):
        ret = super().schedule_and_allocate()
        self._split_multi_waits()
        return ret

    def _split_multi_waits(self):
        nc = self.nc
        for fn in nc.m.functions:
            for bb in fn.blocks:
                if not any(
                    inst.sync_info
                    and inst.sync_info.on_wait
                    and len(inst.sync_info.on_wait) > 1
                    for inst in bb.instructions
                ):
                    continue
                new_insts = []
                for inst in bb.instructions:
                    si = inst.sync_info
                    waits = list(si.on_wait) if si and si.on_wait else []
                    if len(waits) > 1:
                        si.on_wait = waits[-1:]
                        for w in waits[:-1]:
                            nop = mybir.InstNoOp(
                                name=f"I-splitw-{nc.next_id()}",
                                engine=inst.engine,
                                sync_info=mybir.SyncInfo(on_wait=[w], on_update=[]),
                            )
                            nc.register_instruction(nop, overwrite=True)
                            new_insts.append(nop)
                    new_insts.append(inst)
                bb.instructions[:] = new_insts


def _build():
    nc = bass.Bass("TRN2", target_bir_lowering=False, debug=False)
    x = nc.dram_tensor("x", [ROWS, C], F32, kind="ExternalInput").ap()
    wv = nc.dram_tensor("wv", [TAPS], F32, kind="ExternalInput").ap()
    y = nc.dram_tensor("y", [GROUPS, C], F32, kind="ExternalOutput").ap()

    with _TileContext(nc) as tc:
        with ExitStack() as ctx:
            xin = ctx.enter_context(tc.tile_pool(name="xin", bufs=4))
            kp = ctx.enter_context(tc.tile_pool(name="kp", bufs=3))
            sp = ctx.enter_context(tc.tile_pool(name="sp", bufs=1))
            op = ctx.enter_context(tc.tile_pool(name="op", bufs=2))

            wrep = sp.tile([128, TAPS], F32)
            nc.sync.dma_start(wrep[:], wv[None, :].broadcast_to([128, TAPS]))
            acc_all = sp.tile([128, NQ], F32)

            for q in range(NQ):
                xt = xin.tile([128, FD], F32, tag="xt")
                nc.sync.dma_start(
                    xt[:],
                    x[q * GROUP_ROWS : (q + 1) * GROUP_ROWS, :].rearrange(
                        "(p k) c -> p (k c)", k=TAPS
                    ),
                )
                sk = kp.tile([128, TAPS], F32, tag="sk")
                nc.vector.reduce_sum(
                    sk[:],
                    xt.rearrange("p (k c) -> p k c", c=C),
                    axis=mybir.AxisListType.X,
                )
                skw = kp.tile([128, TAPS], F32, tag="skw")
                nc.vector.tensor_mul(skw[:], sk[:], wrep[:])
                nc.vector.reduce_sum(
                    acc_all[:, q : q + 1], skw[:], axis=mybir.AxisListType.X
                )

            for o in range(NQ // OBLK):
                osb = op.tile([128, OBLK * C], F32, tag="osb")
                for qq in range(OBLK):
                    qg = o * OBLK + qq
                    nc.scalar.activation(
                        osb[:, qq * C : (qq + 1) * C],
                        acc_all[:, qg : qg + 1].broadcast_to([128, C]),
                        mybir.ActivationFunctionType.Identity,
                    )
                nc.scalar.dma_start(
                    y[o * OBLK * 128 : (o + 1) * OBLK * 128, :].rearrange(
                        "(q p) c -> p q c", p=128
                    ),
                    osb.rearrange("p (q c) -> p q c", c=C),
                )
    return nc


_NC_CACHE = {}


def _get_nc():
    if "nc" not in _NC_CACHE:
        _NC_CACHE["nc"] = _build()
    return _NC_CACHE["nc"]


def _tap_weights(param3: float, param4: float) -> np.ndarray:
    i = np.arange(1, TAPS + 1, dtype=np.float32)
    logits = (np.float32(param3) * i + np.float32(param4) * i * i).astype(np.float32)
    e = np.exp(logits - logits.max(), dtype=np.float32)
    return (e / e.sum()).astype(np.float32)  # [TAPS]


def run_with_results(inputs, **spmd_kwargs):
    x = np.ascontiguousarray(np.asarray(inputs["inputs"], dtype=np.float32))
    assert x.shape == (B, L, C), x.shape
    wv = _tap_weights(
        float(np.asarray(inputs["param3"])), float(np.asarray(inputs["param4"]))
    )
    xs = x.reshape(NCORES, ROWS, C)
    in_maps = [{"x": xs[i], "wv": wv} for i in range(NCORES)]
    res = run_bass_kernel_spmd(_get_nc(), in_maps, list(range(NCORES)), **spmd_kwargs)
    out = np.stack([res.results[i]["y"] for i in range(NCORES)])
    return out.reshape(B, T, C).astype(np.float32, copy=False), res


def kernel(**inputs) -> np.ndarray:
    out, _ = run_with_results(inputs)
    return out
